# revision 1
# baseline (speedup 1.0000x reference)
"""Trainium2 Bass kernel for the CIN (xDeepFM) block.

inputs [2048,39,16] f32, W0 [1521,128], W1 [4992,128] -> out [2048,256] f32.
Data-parallel over the batch axis across 8 NeuronCores; weights replicated.

Per-core: layer-0 z is built on the PE via summed selection matmuls and the
polarization identity (squares applied during the ScalarE PSUM evacuation);
layer-1 contracts the embedding dim first (per-batch Gram H via one K=128
matmul per chunk against a block-diagonal X0, whose extra one-hot columns
also yield the out1 d-sums); out2 accumulates H^T-slices against W1.
All matmul operands bf16, accumulation fp32.
"""


import ml_dtypes
import numpy as np

BF16 = ml_dtypes.bfloat16

B, M0, D = 2048, 39, 16
C0, C1 = 128, 128
NCORES = 8
BL = B // NCORES          # 256 batches per core
R = BL * D                # 4096 rows per core
NPAIR = (M0 * (M0 + 1)) // 2   # 819
KT = (NPAIR + 127) // 128      # 7 K-tiles
KPAD = KT * 128                # 896
RC = 512                       # r-chunk for z-build
NRC = R // RC                  # 8
NCHUNK = R // 128              # 32 chunks of (8 b x 16 d)
BPC = 128 // D                 # 8 batches per 128-row chunk


def host_constants(W0, W1):
    """Core-independent prepped tensors.

    Square trick: x_m*x_n = 0.5*(x_m+x_n)^2 - 0.5*x_m^2 - 0.5*x_n^2, so
      X1[r,:] = sum_p (0.5*W0sym[p])*sq_p[r] + sum_m Ah[m]*xsq[m,r]
    with sq_p = (x_m(p)+x_n(p))^2 built on the PE via the summed selection
    matrix and squared during the ACT PSUM->SBUF evacuation.
    """
    pairs = [(m, n) for m in range(M0) for n in range(m, M0)]
    assert len(pairs) == NPAIR

    selsum = np.zeros((128, KT, 128), dtype=np.float32)
    for p, (m, n) in enumerate(pairs):
        t, q = divmod(p, 128)
        selsum[m, t, q] += 1.0
        selsum[n, t, q] += 1.0
    # pseudo-pairs in the spare slots: s = x_m so sq = x_m^2, carrying the
    # -0.5*(x_m^2+x_n^2) correction terms with weights Ah
    for m in range(M0):
        t, q = divmod(NPAIR + m, 128)
        selsum[m, t, q] = 1.0

    W0r = W0.reshape(M0, M0, C0)
    w0sym = np.zeros((KPAD, C0), dtype=np.float32)
    for p, (m, n) in enumerate(pairs):
        if m == n:
            w0sym[p] = W0r[m, m]
        else:
            w0sym[p] = W0r[m, n] + W0r[n, m]
    # half factor folded into the sq-contraction weights
    w0h_kt = 0.5 * w0sym.reshape(KT, 128, C0).transpose(1, 0, 2).copy()

    # correction weights for the -0.5*(x_m^2 + x_n^2) terms, placed in the
    # same spare slots (note w0h rows are 0.5*w0sym, so store 2*Ah... no:
    # w0h_kt is built from w0sym then halved; patch the rows directly after)
    Ah = np.zeros((M0, C0), dtype=np.float32)
    for p, (m, n) in enumerate(pairs):
        Ah[m] -= 0.5 * w0sym[p]
        Ah[n] -= 0.5 * w0sym[p]
    for m in range(M0):
        t, q = divmod(NPAIR + m, 128)
        w0h_kt[q, t, :] = Ah[m]

    # w1sb[n, m, o] = W1[m*128+n, o]
    w1sb = W1.reshape(M0, C1, C0).transpose(1, 0, 2).copy()

    return dict(
        selsum=np.ascontiguousarray(selsum.astype(BF16)),
        w0h=np.ascontiguousarray(w0h_kt.astype(BF16)),
        w1sb=np.ascontiguousarray(w1sb.astype(BF16)),
        ident=np.ascontiguousarray(np.eye(128, dtype=np.float32).astype(BF16)),
    )


def host_core_inputs(x_c):
    """Per-core prepped tensors from the [BL, M0, D] input shard."""
    xdT = np.zeros((128, R), dtype=np.float32)
    # xdT[m, b*D+d] = x[b, m, d]
    xdT[:M0] = x_c.transpose(1, 0, 2).reshape(M0, R)
    # full block-diagonal Gram operand, zero-padded on the host:
    # xT2z[(b8, d), ch, b8'*M0+m] = x[ch*8+b8, m, d] if b8 == b8' else 0
    xtt = x_c.reshape(NCHUNK, BPC, M0, D).transpose(1, 3, 0, 2)  # [b8, d, ch, m]
    xT2z = np.zeros((BPC, D, NCHUNK, BPC, M0 + 1), dtype=np.float32)
    for b8 in range(BPC):
        xT2z[b8, :, :, b8, :M0] = xtt[b8]
        xT2z[b8, :, :, b8, M0] = 1.0  # d-sum -> out1^T column
    xT2z = xT2z.reshape(128, NCHUNK, BPC * (M0 + 1))
    return dict(
        xdT=np.ascontiguousarray(xdT.astype(BF16)),
        xt=np.ascontiguousarray(xT2z.astype(BF16)),
    )


def apply_tile_patch():
    """walrus in this toolchain rejects >1 sync-wait per instruction; no-op
    here — split_sync_waits() post-processes the whole program instead."""


def split_sync_waits(nc):
    """Rewrite every instruction carrying >1 sync wait: keep the first wait,
    hoist the rest onto same-engine NoOps inserted immediately before it."""
    import concourse.mybir as mybir

    counter = [0]
    for f in nc.m.functions:
        for bb in f.blocks:
            new_list = []
            changed = False
            for inst in bb.instructions:
                si = inst.sync_info
                waits = list(si.on_wait) if si is not None else []
                if len(waits) > 1:
                    changed = True
                    for w in waits[:-1]:
                        counter[0] += 1
                        nop = mybir.InstNoOp(
                            name=f"WSPLIT-{counter[0]}", ins=[], outs=[]
                        )
                        nop.engine = inst.engine
                        nop.sync_info = mybir.SyncInfo(on_wait=[w], on_update=[])
                        new_list.append(nop)
                    si.on_wait = waits[-1:]
                new_list.append(inst)
            if changed:
                bb.instructions = new_list
    return counter[0]


def build_program(reps=1, split_waits=True, loop_reps=None, cfg=None):
    """loop_reps: if set, wrap the whole body in a tc.For_i hardware loop with
    that trip count (for slope-based HW timing)."""
    import contextlib

    cfg = cfg or {}
    PAIRING = cfg.get("pairing", True)
    SEL2_BUFS = cfg.get("sel2", 2)
    SEL1_BUFS = cfg.get("sel1", 1)
    X1_BUFS = cfg.get("x1", 2)
    H_BUFS = cfg.get("h", 1)
    SQ_BUFS = cfg.get("sq", 2)
    XSPLIT = cfg.get("xsplit", 4)
    TSPLIT = cfg.get("tsplit", 4)
    CBUFS = cfg.get("cbufs", 2)
    UNROLL = cfg.get("unroll", 2)

    import concourse.bass as bass
    import concourse.mybir as mybir
    import concourse.tile as tile

    f32 = mybir.dt.float32
    bf16 = mybir.dt.bfloat16

    nc = bass.Bass("TRN2", target_bir_lowering=False, debug=False)
    d_xdT = nc.dram_tensor("xdT", [128, R], bf16, kind="ExternalInput")
    HW_ = BPC * (M0 + 1)
    d_xt = nc.dram_tensor("xt", [128, NCHUNK, HW_], bf16, kind="ExternalInput")
    d_id = nc.dram_tensor("ident", [128, 128], bf16, kind="ExternalInput")
    d_sel = nc.dram_tensor("selsum", [128, KT, 128], bf16, kind="ExternalInput")
    d_w0 = nc.dram_tensor("w0h", [128, KT, C0], bf16, kind="ExternalInput")
    d_w1 = nc.dram_tensor("w1sb", [128, M0, C0], bf16, kind="ExternalInput")
    d_out = nc.dram_tensor("out", [BL, C0 + C1], f32, kind="ExternalOutput")

    with tile.TileContext(nc) as tc:
        if loop_reps is not None:
            # 2x-unrolled body with rotating input buffers so the next
            # iteration's DMAs overlap this iteration's compute
            assert loop_reps % UNROLL == 0
            trips = loop_reps // UNROLL
            reps = UNROLL
        else:
            trips = None
        with (
            tc.tile_pool(name="const", bufs=CBUFS) as cpool,
            tc.tile_pool(name="x1sb", bufs=1) as x1pool,
            tc.tile_pool(name="hsb", bufs=1) as hpool,
            tc.tile_pool(name="sq", bufs=SQ_BUFS) as sqpool,
            tc.tile_pool(name="outp", bufs=1) as opool,
            tc.tile_pool(name="ps_x1", bufs=X1_BUFS, space="PSUM") as ps_x1,
            tc.tile_pool(name="ps_h", bufs=H_BUFS, space="PSUM") as ps_h,
            tc.tile_pool(name="ps_sel", bufs=SEL2_BUFS, space="PSUM") as ps_sel,
            tc.tile_pool(name="ps_sel1", bufs=SEL1_BUFS, space="PSUM") as ps_sel1,
        ):
            loop_cm = (
                tc.For_i(
                    0,
                    trips,
                    1,
                    hint_engines=(
                        mybir.EngineType.PE,
                        mybir.EngineType.Activation,
                        mybir.EngineType.DVE,
                        mybir.EngineType.SP,
                    ),
                )
                if loop_reps is not None
                else contextlib.nullcontext()
            )
            with loop_cm:
                for _rep in range(reps):
                    xdT = cpool.tile([128, R], bf16, tag="xdT")
                    sel = cpool.tile([128, KT, 128], bf16, tag="sel")
                    w0 = cpool.tile([128, KT, C0], bf16, tag="w0")
                    w1 = cpool.tile([128, M0, C0], bf16, tag="w1")
                    xT2z = cpool.tile([128, NCHUNK, HW_], bf16, tag="xT2z")
                    ident = cpool.tile([128, 128], bf16, tag="ident")
                    # first matmul's operands first
                    nc.sync.dma_start(xdT[:, : R // XSPLIT], d_xdT[:, : R // XSPLIT])
                    nc.sync.dma_start(sel[:], d_sel[:, :, :])
                    nc.sync.dma_start(w0[:], d_w0[:, :, :])
                    for q in range(1, XSPLIT):
                        nc.sync.dma_start(
                            xdT[:, q * (R // XSPLIT) : (q + 1) * (R // XSPLIT)],
                            d_xdT[:, q * (R // XSPLIT) : (q + 1) * (R // XSPLIT)],
                        )
                    for q in range(TSPLIT):
                        cs = slice(q * (NCHUNK // TSPLIT), (q + 1) * (NCHUNK // TSPLIT))
                        nc.sync.dma_start(xT2z[:, cs, :], d_xt[:, cs, :])
                    nc.sync.dma_start(w1[:], d_w1[:, :, :])
                    nc.sync.dma_start(ident[:], d_id[:, :])

                    x1sb = x1pool.tile([128, NCHUNK, C0], bf16, tag="x1sb")
                    hsb = hpool.tile([128, NCHUNK, HW_], bf16, tag="hsb")

                    # ---- stage 1: squares of pair-sums + X1 ----
                    hsb3 = hsb[:].rearrange("n c (b m) -> n (c b) m", m=M0 + 1)
                    outsb = opool.tile([128, 2, C0 + C1], f32, tag="outsb")

                    def emit_out(bt):
                        o2_ps = ps_x1.tile([128, C1], f32, tag="x1")
                        for m in range(M0):
                            nc.tensor.matmul(
                                o2_ps[:],
                                hsb3[:, bt * 128 : (bt + 1) * 128, m],
                                w1[:, m, :],
                                start=(m == 0),
                                stop=(m == M0 - 1),
                            )
                        nc.vector.tensor_copy(outsb[:, bt, C0:], o2_ps[:])
                        # out1^T columns (d-sums from the extended H matmul),
                        # flipped to [b, o] with a PE transpose
                        o1_ps = ps_sel1.tile([128, C0], bf16, tag="sum1")
                        nc.tensor.transpose(
                            o1_ps[:],
                            hsb3[:, bt * 128 : (bt + 1) * 128, M0],
                            ident[:],
                        )
                        nc.vector.tensor_copy(outsb[:, bt, :C0], o1_ps[:])
                        nc.sync.dma_start(
                            d_out[bt * 128 : (bt + 1) * 128, :], outsb[:, bt, :]
                        )

                    # K-tile pairing: 3 double-width square evacs + 1 single
                    TP = [(0, 1), (2, 3), (4, 5), (6,)] if PAIRING else [
                        (0,), (1,), (2,), (3,), (4,), (5,), (6,)]
                    for rc in range(NRC):
                        rsl = slice(rc * RC, (rc + 1) * RC)
                        sqs = [None] * KT
                        for grp in TP:
                            pool = ps_sel if len(grp) == 2 else ps_sel1
                            sum_ps = pool.tile(
                                [128, len(grp), RC], f32, tag=f"sum{len(grp)}"
                            )
                            for j, t in enumerate(grp):
                                nc.tensor.matmul(
                                    sum_ps[:, j, :],
                                    sel[:, t, :],
                                    xdT[:, rsl],
                                    start=True,
                                    stop=True,
                                )
                            sq = sqpool.tile(
                                [128, len(grp), RC], bf16, tag=f"sq{grp[0]}"
                            )
                            nc.scalar.square(sq[:], sum_ps[:])
                            for j, t in enumerate(grp):
                                sqs[t] = sq[:, j, :]
                        for rs in range(RC // 128):
                            ch = rc * (RC // 128) + rs
                            csl = slice(rs * 128, (rs + 1) * 128)
                            x1_ps = ps_x1.tile([128, C0], f32, tag="x1")
                            for t in range(KT):
                                nc.tensor.matmul(
                                    x1_ps[:],
                                    sqs[t][:, csl],
                                    w0[:, t, :],
                                    start=(t == 0),
                                    stop=(t == KT - 1),
                                )
                            nc.vector.tensor_copy(x1sb[:, ch, :], x1_ps[:])
                            # all 8 per-batch Grams of this chunk in one
                            # K=128 matmul vs the block-diagonal x
                            h_ps = ps_h.tile([128, HW_], f32, tag="h")
                            nc.tensor.matmul(
                                h_ps[:],
                                x1sb[:, ch, :],
                                xT2z[:, ch, :],
                                start=True,
                                stop=True,
                            )
                            nc.vector.tensor_copy(hsb[:, ch, :], h_ps[:])
                            if ch == 15:
                                emit_out(0)
                            elif ch == 31:
                                emit_out(1)

    if split_waits:
        split_sync_waits(nc)
    return nc


def make_in_maps(inputs, W0, W1):
    consts = host_constants(np.asarray(W0), np.asarray(W1))
    in_maps = []
    for c in range(NCORES):
        x_c = np.ascontiguousarray(np.asarray(inputs)[c * BL : (c + 1) * BL])
        m = dict(consts)
        m.update(host_core_inputs(x_c))
        in_maps.append(m)
    return in_maps


_KERNEL_CACHE = {}


def kernel(inputs, W0, W1):
    inputs = np.ascontiguousarray(np.asarray(inputs, dtype=np.float32))
    W0 = np.ascontiguousarray(np.asarray(W0, dtype=np.float32))
    W1 = np.ascontiguousarray(np.asarray(W1, dtype=np.float32))
    assert inputs.shape == (B, M0, D) and W0.shape == (M0 * M0, C0)
    assert W1.shape == (M0 * C0, C1)

    if "nc" not in _KERNEL_CACHE:
        _KERNEL_CACHE["nc"] = build_program()
    nc = _KERNEL_CACHE["nc"]

    consts = host_constants(W0, W1)
    in_maps = []
    for c in range(NCORES):
        m = dict(consts)
        m.update(host_core_inputs(inputs[c * BL : (c + 1) * BL]))
        in_maps.append(m)

    from concourse.bass_utils import run_bass_kernel_spmd

    res = run_bass_kernel_spmd(nc, in_maps, core_ids=list(range(NCORES)))
    out = np.concatenate([res.results[c]["out"] for c in range(NCORES)], axis=0)
    return np.ascontiguousarray(out.astype(np.float32))



# revision 2
# speedup vs baseline: 1.3944x; 1.3944x over previous
"""Trainium2 Bass kernel for the CIN (xDeepFM) block.

inputs [2048,39,16] f32, W0 [1521,128], W1 [4992,128] -> out [2048,256] f32.
Data-parallel over the batch axis across 8 NeuronCores; weights replicated.

Per-core: layer-0 z is built on the PE via summed selection matmuls and the
polarization identity (squares applied during the ScalarE PSUM evacuation);
layer-1 contracts the embedding dim first (per-batch Gram H via one K=128
matmul per chunk against a block-diagonal X0, whose extra one-hot columns
also yield the out1 d-sums); out2 accumulates H^T-slices against W1.
All matmul operands bf16, accumulation fp32.
"""


import ml_dtypes
import numpy as np

BF16 = ml_dtypes.bfloat16

B, M0, D = 2048, 39, 16
C0, C1 = 128, 128
NCORES = 8
BL = B // NCORES          # 256 batches per core
R = BL * D                # 4096 rows per core
NPAIR = (M0 * (M0 + 1)) // 2   # 819
KT = (NPAIR + 127) // 128      # 7 K-tiles
KPAD = KT * 128                # 896
RC = 512                       # r-chunk for z-build
NRC = R // RC                  # 8
NCHUNK = R // 128              # 32 chunks of (8 b x 16 d)
BPC = 128 // D                 # 8 batches per 128-row chunk


def host_constants(W0, W1):
    """Core-independent prepped tensors.

    Square trick: x_m*x_n = 0.5*(x_m+x_n)^2 - 0.5*x_m^2 - 0.5*x_n^2, so
      X1[r,:] = sum_p (0.5*W0sym[p])*sq_p[r] + sum_m Ah[m]*xsq[m,r]
    with sq_p = (x_m(p)+x_n(p))^2 built on the PE via the summed selection
    matrix and squared during the ACT PSUM->SBUF evacuation.
    """
    pairs = [(m, n) for m in range(M0) for n in range(m, M0)]
    assert len(pairs) == NPAIR

    selsum = np.zeros((128, KT, 128), dtype=np.float32)
    for p, (m, n) in enumerate(pairs):
        t, q = divmod(p, 128)
        base = 64 * (t % 2)
        selsum[base + m, t, q] += 1.0
        selsum[base + n, t, q] += 1.0
    # pseudo-pairs in the spare slots: s = x_m so sq = x_m^2, carrying the
    # -0.5*(x_m^2+x_n^2) correction terms with weights Ah
    for m in range(M0):
        t, q = divmod(NPAIR + m, 128)
        selsum[64 * (t % 2) + m, t, q] = 1.0

    W0r = W0.reshape(M0, M0, C0)
    w0sym = np.zeros((KPAD, C0), dtype=np.float32)
    for p, (m, n) in enumerate(pairs):
        if m == n:
            w0sym[p] = W0r[m, m]
        else:
            w0sym[p] = W0r[m, n] + W0r[n, m]
    # half factor folded into the sq-contraction weights
    w0h_kt = 0.5 * w0sym.reshape(KT, 128, C0).transpose(1, 0, 2).copy()

    # correction weights for the -0.5*(x_m^2 + x_n^2) terms, placed in the
    # same spare slots (note w0h rows are 0.5*w0sym, so store 2*Ah... no:
    # w0h_kt is built from w0sym then halved; patch the rows directly after)
    Ah = np.zeros((M0, C0), dtype=np.float32)
    for p, (m, n) in enumerate(pairs):
        Ah[m] -= 0.5 * w0sym[p]
        Ah[n] -= 0.5 * w0sym[p]
    for m in range(M0):
        t, q = divmod(NPAIR + m, 128)
        w0h_kt[q, t, :] = Ah[m]

    # w1sb[n, m, o] = W1[m*128+n, o]
    w1sb = W1.reshape(M0, C1, C0).transpose(1, 0, 2).copy()

    return dict(
        selsum=np.ascontiguousarray(selsum.astype(BF16)),
        w0h=np.ascontiguousarray(w0h_kt.astype(BF16)),
        w1sb=np.ascontiguousarray(w1sb.astype(BF16)),
        ident=np.ascontiguousarray(np.eye(128, dtype=np.float32).astype(BF16)),
    )


def host_core_inputs(x_c):
    """Per-core prepped tensors from the [BL, M0, D] input shard."""
    xdT = np.zeros((128, R), dtype=np.float32)
    # xdT[m, b*D+d] = x[b, m, d], replicated at partition base 64 so
    # consecutive K=39 selection matmuls row-pack 2-way on the PE
    xdT[:M0] = x_c.transpose(1, 0, 2).reshape(M0, R)
    xdT[64 : 64 + M0] = xdT[:M0]
    # full block-diagonal Gram operand, zero-padded on the host:
    # xT2z[(b8, d), ch, b8'*M0+m] = x[ch*8+b8, m, d] if b8 == b8' else 0
    xtt = x_c.reshape(NCHUNK, BPC, M0, D).transpose(1, 3, 0, 2)  # [b8, d, ch, m]
    xT2z = np.zeros((BPC, D, NCHUNK, BPC, M0 + 1), dtype=np.float32)
    for b8 in range(BPC):
        xT2z[b8, :, :, b8, :M0] = xtt[b8]
        xT2z[b8, :, :, b8, M0] = 1.0  # d-sum -> out1^T column
    xT2z = xT2z.reshape(128, NCHUNK, BPC * (M0 + 1))
    return dict(
        xdT=np.ascontiguousarray(xdT.astype(BF16)),
        xt=np.ascontiguousarray(xT2z.astype(BF16)),
    )


def apply_tile_patch():
    """walrus in this toolchain rejects >1 sync-wait per instruction; no-op
    here — split_sync_waits() post-processes the whole program instead."""


def split_sync_waits(nc):
    """Rewrite every instruction carrying >1 sync wait: keep the first wait,
    hoist the rest onto same-engine NoOps inserted immediately before it."""
    import concourse.mybir as mybir

    counter = [0]
    for f in nc.m.functions:
        for bb in f.blocks:
            new_list = []
            changed = False
            for inst in bb.instructions:
                si = inst.sync_info
                waits = list(si.on_wait) if si is not None else []
                if len(waits) > 1:
                    changed = True
                    for w in waits[:-1]:
                        counter[0] += 1
                        nop = mybir.InstNoOp(
                            name=f"WSPLIT-{counter[0]}", ins=[], outs=[]
                        )
                        nop.engine = inst.engine
                        nop.sync_info = mybir.SyncInfo(on_wait=[w], on_update=[])
                        new_list.append(nop)
                    si.on_wait = waits[-1:]
                new_list.append(inst)
            if changed:
                bb.instructions = new_list
    return counter[0]


def build_program(reps=1, split_waits=True, loop_reps=None, cfg=None):
    """loop_reps: if set, wrap the whole body in a tc.For_i hardware loop with
    that trip count (for slope-based HW timing)."""
    import contextlib

    cfg = cfg or {}
    PAIRING = cfg.get("pairing", True)
    SEL2_BUFS = cfg.get("sel2", 2)
    SEL1_BUFS = cfg.get("sel1", 1)
    X1_BUFS = cfg.get("x1", 2)
    H_BUFS = cfg.get("h", 1)
    SQ_BUFS = cfg.get("sq", 2)
    XSPLIT = cfg.get("xsplit", 4)
    TSPLIT = cfg.get("tsplit", 4)
    CBUFS = cfg.get("cbufs", 2)
    UNROLL = cfg.get("unroll", 2)

    import concourse.bass as bass
    import concourse.mybir as mybir
    import concourse.tile as tile

    f32 = mybir.dt.float32
    bf16 = mybir.dt.bfloat16

    nc = bass.Bass("TRN2", target_bir_lowering=False, debug=False)
    d_xdT = nc.dram_tensor("xdT", [128, R], bf16, kind="ExternalInput")
    HW_ = BPC * (M0 + 1)
    d_xt = nc.dram_tensor("xt", [128, NCHUNK, HW_], bf16, kind="ExternalInput")
    d_id = nc.dram_tensor("ident", [128, 128], bf16, kind="ExternalInput")
    d_sel = nc.dram_tensor("selsum", [128, KT, 128], bf16, kind="ExternalInput")
    d_w0 = nc.dram_tensor("w0h", [128, KT, C0], bf16, kind="ExternalInput")
    d_w1 = nc.dram_tensor("w1sb", [128, M0, C0], bf16, kind="ExternalInput")
    d_out = nc.dram_tensor("out", [BL, C0 + C1], f32, kind="ExternalOutput")

    with tile.TileContext(nc) as tc:
        if loop_reps is not None:
            # 2x-unrolled body with rotating input buffers so the next
            # iteration's DMAs overlap this iteration's compute
            assert loop_reps % UNROLL == 0
            trips = loop_reps // UNROLL
            reps = UNROLL
        else:
            trips = None
        with (
            tc.tile_pool(name="const", bufs=CBUFS) as cpool,
            tc.tile_pool(name="x1sb", bufs=1) as x1pool,
            tc.tile_pool(name="hsb", bufs=1) as hpool,
            tc.tile_pool(name="sq", bufs=SQ_BUFS) as sqpool,
            tc.tile_pool(name="outp", bufs=1) as opool,
            tc.tile_pool(name="ps_x1", bufs=X1_BUFS, space="PSUM") as ps_x1,
            tc.tile_pool(name="ps_h", bufs=H_BUFS, space="PSUM") as ps_h,
            tc.tile_pool(name="ps_sel", bufs=SEL2_BUFS, space="PSUM") as ps_sel,
            tc.tile_pool(name="ps_sel1", bufs=SEL1_BUFS, space="PSUM") as ps_sel1,
        ):
            loop_cm = (
                tc.For_i(
                    0,
                    trips,
                    1,
                    hint_engines=(
                        mybir.EngineType.PE,
                        mybir.EngineType.Activation,
                        mybir.EngineType.DVE,
                        mybir.EngineType.SP,
                    ),
                )
                if loop_reps is not None
                else contextlib.nullcontext()
            )
            with loop_cm:
                for _rep in range(reps):
                    xdT = cpool.tile([128, R], bf16, tag="xdT")
                    sel = cpool.tile([128, KT, 128], bf16, tag="sel")
                    w0 = cpool.tile([128, KT, C0], bf16, tag="w0")
                    w1 = cpool.tile([128, M0, C0], bf16, tag="w1")
                    xT2z = cpool.tile([128, NCHUNK, HW_], bf16, tag="xT2z")
                    ident = cpool.tile([128, 128], bf16, tag="ident")
                    # first matmul's operands first
                    nc.sync.dma_start(xdT[:, : R // XSPLIT], d_xdT[:, : R // XSPLIT])
                    nc.sync.dma_start(sel[:], d_sel[:, :, :])
                    nc.sync.dma_start(w0[:], d_w0[:, :, :])
                    for q in range(1, XSPLIT):
                        nc.sync.dma_start(
                            xdT[:, q * (R // XSPLIT) : (q + 1) * (R // XSPLIT)],
                            d_xdT[:, q * (R // XSPLIT) : (q + 1) * (R // XSPLIT)],
                        )
                    for q in range(TSPLIT):
                        cs = slice(q * (NCHUNK // TSPLIT), (q + 1) * (NCHUNK // TSPLIT))
                        nc.sync.dma_start(xT2z[:, cs, :], d_xt[:, cs, :])
                    nc.sync.dma_start(w1[:], d_w1[:, :, :])
                    nc.sync.dma_start(ident[:], d_id[:, :])

                    x1sb = x1pool.tile([128, NCHUNK, C0], bf16, tag="x1sb")
                    hsb = hpool.tile([128, NCHUNK, HW_], bf16, tag="hsb")

                    # ---- stage 1: squares of pair-sums + X1 ----
                    hsb3 = hsb[:].rearrange("n c (b m) -> n (c b) m", m=M0 + 1)
                    outsb = opool.tile([128, 2, C0 + C1], f32, tag="outsb")

                    def emit_out(bt):
                        o2_ps = ps_x1.tile([128, C1], f32, tag="x1")
                        for m in range(M0):
                            nc.tensor.matmul(
                                o2_ps[:],
                                hsb3[:, bt * 128 : (bt + 1) * 128, m],
                                w1[:, m, :],
                                start=(m == 0),
                                stop=(m == M0 - 1),
                            )
                        nc.vector.tensor_copy(outsb[:, bt, C0:], o2_ps[:])
                        # out1^T columns (d-sums from the extended H matmul),
                        # flipped to [b, o] with a PE transpose
                        o1_ps = ps_sel1.tile([128, C0], bf16, tag="sum1")
                        nc.tensor.transpose(
                            o1_ps[:],
                            hsb3[:, bt * 128 : (bt + 1) * 128, M0],
                            ident[:],
                        )
                        nc.vector.tensor_copy(outsb[:, bt, :C0], o1_ps[:])
                        nc.sync.dma_start(
                            d_out[bt * 128 : (bt + 1) * 128, :], outsb[:, bt, :]
                        )

                    # K-tile pairing: 3 double-width square evacs + 1 single
                    TP = [(0, 1), (2, 3), (4, 5), (6,)] if PAIRING else [
                        (0,), (1,), (2,), (3,), (4,), (5,), (6,)]
                    for rc in range(NRC):
                        rsl = slice(rc * RC, (rc + 1) * RC)
                        sqs = [None] * KT
                        for grp in TP:
                            pool = ps_sel if len(grp) == 2 else ps_sel1
                            sum_ps = pool.tile(
                                [128, len(grp), RC], f32, tag=f"sum{len(grp)}"
                            )
                            for j, t in enumerate(grp):
                                base = 64 * (t % 2)
                                nc.tensor.matmul(
                                    sum_ps[:, j, :],
                                    sel[base : base + M0, t, :],
                                    xdT[base : base + M0, rsl],
                                    start=True,
                                    stop=True,
                                    tile_position=(base, 0),
                                )
                            sq = sqpool.tile(
                                [128, len(grp), RC], bf16, tag=f"sq{grp[0]}"
                            )
                            nc.scalar.square(sq[:], sum_ps[:])
                            for j, t in enumerate(grp):
                                sqs[t] = sq[:, j, :]
                        for rs in range(RC // 128):
                            ch = rc * (RC // 128) + rs
                            csl = slice(rs * 128, (rs + 1) * 128)
                            x1_ps = ps_x1.tile([128, C0], f32, tag="x1")
                            for t in range(KT):
                                nc.tensor.matmul(
                                    x1_ps[:],
                                    sqs[t][:, csl],
                                    w0[:, t, :],
                                    start=(t == 0),
                                    stop=(t == KT - 1),
                                )
                            nc.vector.tensor_copy(x1sb[:, ch, :], x1_ps[:])
                            # all 8 per-batch Grams of this chunk in one
                            # K=128 matmul vs the block-diagonal x
                            h_ps = ps_h.tile([128, HW_], f32, tag="h")
                            nc.tensor.matmul(
                                h_ps[:],
                                x1sb[:, ch, :],
                                xT2z[:, ch, :],
                                start=True,
                                stop=True,
                            )
                            nc.vector.tensor_copy(hsb[:, ch, :], h_ps[:])
                            if ch == 15:
                                emit_out(0)
                            elif ch == 31:
                                emit_out(1)

    if split_waits:
        split_sync_waits(nc)
    return nc


def make_in_maps(inputs, W0, W1):
    consts = host_constants(np.asarray(W0), np.asarray(W1))
    in_maps = []
    for c in range(NCORES):
        x_c = np.ascontiguousarray(np.asarray(inputs)[c * BL : (c + 1) * BL])
        m = dict(consts)
        m.update(host_core_inputs(x_c))
        in_maps.append(m)
    return in_maps


_KERNEL_CACHE = {}


def kernel(inputs, W0, W1):
    inputs = np.ascontiguousarray(np.asarray(inputs, dtype=np.float32))
    W0 = np.ascontiguousarray(np.asarray(W0, dtype=np.float32))
    W1 = np.ascontiguousarray(np.asarray(W1, dtype=np.float32))
    assert inputs.shape == (B, M0, D) and W0.shape == (M0 * M0, C0)
    assert W1.shape == (M0 * C0, C1)

    if "nc" not in _KERNEL_CACHE:
        _KERNEL_CACHE["nc"] = build_program()
    nc = _KERNEL_CACHE["nc"]

    consts = host_constants(W0, W1)
    in_maps = []
    for c in range(NCORES):
        m = dict(consts)
        m.update(host_core_inputs(inputs[c * BL : (c + 1) * BL]))
        in_maps.append(m)

    from concourse.bass_utils import run_bass_kernel_spmd

    res = run_bass_kernel_spmd(nc, in_maps, core_ids=list(range(NCORES)))
    out = np.concatenate([res.results[c]["out"] for c in range(NCORES)], axis=0)
    return np.ascontiguousarray(out.astype(np.float32))

